# revision 26
# baseline (speedup 1.0000x reference)
"""Causal attention B=4 S=4096 D=64 on 8 TRN2 NeuronCores.

Sharding: core c -> batch b = c//2, type = c%2. Each core owns 8 q-tiles of
256 rows: type A takes tiles i=2t, type B i=2t+1 (t=0..7), so every core's
causal work is ~balanced and the compiled program is identical across cores
(slot t processes cap_t = 4t+4 k-blocks of 128 rows; cores whose tile needs
fewer blocks have the excess masked to zero via per-core mask data).

Device algorithm per (slot t, group g of 4 k-blocks):
  S^T[k,q] = K_blk @ Q^T  (two K=64-contraction matmuls row-packed via
  partitions 0:64 / 64:128, outputs in different PSUM banks), exp(S^T/8)
  fused on ScalarE PSUM->SBUF bf16, last group multiplied by per-core mask
  (causal triangle / pad zeroing), then out^T[65,256] += V'_blk.T @ P^T
  accumulated in PSUM where V' has a ones column so row 64 = softmax
  denominator. Normalize + transpose on host.
"""

import json

import numpy as np
import ml_dtypes

import concourse.bass as bass
import concourse.mybir as mybir
import concourse.tile as tile
from concourse.bass_utils import run_bass_kernel_spmd
from concourse.vector_clock import ScopedClock

B, S, D = 4, 4096, 64
NCORES = 8
QT = 256               # q-tile width
NSLOT = 8              # q-tiles per core
KB = 128               # k-block rows
SCALE = 1.0 / 8.0      # 1/sqrt(D)
PERM = (0, 2, 1, 3)    # sc position j holds block 4g + PERM[j]

BF16 = mybir.dt.bfloat16
F32 = mybir.dt.float32
npbf16 = ml_dtypes.bfloat16


def _patched_drain_and_barrier(self, tick_clock, wait_clock):
    """The tail Drain may carry N sem waits; this walrus build rejects >1
    wait on CTRL-class instructions, so split them across N drains."""
    drain_inst = self.nc.sync.drain()
    wait_clock.add_sem_waits(
        drain_inst.ins, ScopedClock({None: tick_clock.global_clock})
    )
    si = drain_inst.ins.sync_info
    waits = list(si.on_wait) if si and si.on_wait else []
    if len(waits) > 1:
        si.on_wait = waits[:1]
        for w in waits[1:]:
            d2 = self.nc.sync.drain()
            si2 = d2.ins.sync_info
            if si2 is None:
                d2.ins.sync_info = mybir.SyncInfo(on_wait=[w], on_update=[])
            else:
                si2.on_wait = [w]
    self.nc.all_engine_barrier()
    popped = self.nc._tile_sem_poison_stack.pop()
    assert popped is self._sem_poison
    self.nc.clear_and_free_semaphores(list(self.sems.allocated().values()))
    self.nc.all_engine_barrier()


tile.TileContext._drain_and_barrier = _patched_drain_and_barrier

_orig_to_json_bytes = bass.Bass.to_json_bytes


def _to_json_bytes_split_waits(self) -> bytes:
    """This walrus build accepts at most one sem wait per instruction; spill
    extra waits onto standalone EventSemaphore instructions just before."""
    m = json.loads(_orig_to_json_bytes(self))
    ctr = 0
    for fn in m["functions"]:
        for blk in fn["blocks"]:
            if blk["name"] == "main":
                blk["instructions"] = [
                    i for i in blk["instructions"]
                    if i["opcode"] not in ("Memset", "Drain", "EventSemaphore")
                ]
        for blk in fn["blocks"]:
            out = []
            for inst in blk["instructions"]:
                si = inst.get("sync_info")
                ow = (si or {}).get("on_wait") or []
                if ow:
                    # engines execute their queue in order, so a wait on the
                    # instruction's own engine-completion semaphore is always
                    # already satisfied — drop it (DMA queue sems are named
                    # DMAHW*/DMASW* and never match the engine string).
                    eng = inst.get("engine")
                    kept = [
                        w for w in ow
                        if w.get("ant_name", "").rsplit("_", 1)[0] != eng
                    ]
                    if len(kept) != len(ow):
                        si["on_wait"] = ow = kept
                if len(ow) > 1:
                    for w in ow[:-1]:
                        ctr += 1
                        out.append({
                            "debug": inst.get("debug", 0),
                            "engine": inst["engine"],
                            "ins": [],
                            "outs": [],
                            "name": f"{inst['name']}_sw{ctr}",
                            "opcode": "EventSemaphore",
                            "sync_info": {"on_update": [], "on_wait": [w]},
                        })
                    si["on_wait"] = [ow[-1]]
                out.append(inst)
            blk["instructions"] = out
    return json.dumps(m).encode()


bass.Bass.to_json_bytes = _to_json_bytes_split_waits


def build_nc():
    nc = bass.Bass()
    qT_d = nc.declare_dram_parameter("qT", [128, NSLOT * QT], BF16, isOutput=False)
    kT_d = nc.declare_dram_parameter("kT", [128, 16 * KB], BF16, isOutput=False)
    v_d = nc.declare_dram_parameter("v", [128, 32 * 65], BF16, isOutput=False)
    m_d = nc.declare_dram_parameter("dmask", [128, 4 * QT], BF16, isOutput=False)
    out_d = nc.declare_dram_parameter("outT", [65, NSLOT, QT], F32, isOutput=True)

    with tile.TileContext(nc) as tc:
        with (
            tc.tile_pool(name="const", bufs=1) as cpool,
            tc.tile_pool(name="pT", bufs=6) as ppool,
            tc.tile_pool(name="pTm", bufs=2) as mpool,
            tc.tile_pool(name="osb", bufs=3) as opool,
            tc.tile_pool(name="sc", bufs=3, space="PSUM") as scpool,
            tc.tile_pool(name="acc", bufs=2, space="PSUM") as accpool,
        ):
            # explicit zero bias (avoids the const_aps preamble) + warm
            # the ACT exp table during the DMA prefetch window
            zbias = cpool.tile([128, 1], F32)
            nc.vector.memset(zbias[:], 0.0)
            warm = cpool.tile([128, 8], F32)
            nc.vector.memset(warm[:], 0.0)
            nc.scalar.activation(out=warm[:], in_=warm[:], bias=zbias[:],
                                 func=mybir.ActivationFunctionType.Exp)
            # ~2.6us of back-to-back N=512 zero matmuls inside the DMA window
            # prime the PE clock gate (HAM) so real groups start at 2.4GHz
            wz = cpool.tile([128, 512], BF16)
            nc.vector.memset(wz[:], 0.0)
            wps = accpool.tile([128, 512], F32, tag="acc")
            for _ in range(6):
                nc.tensor.matmul(wps[:], wz[0:64, 0:128], wz[0:64, :],
                                 start=True, stop=True)


            qT_sb = cpool.tile([128, NSLOT, QT], BF16)
            kT_sb = cpool.tile([128, 16, KB], BF16)
            v_sb = cpool.tile([128, 32, 65], BF16)
            mask_sb = cpool.tile([128, 4, QT], BF16)
            qT_dr = qT_d.rearrange("p (s q) -> p s q", q=QT)
            kT_dr = kT_d.rearrange("p (b k) -> p b k", k=KB)
            v_dr = v_d.rearrange("p (b v) -> p b v", v=65)
            # big slots first (descending), slot 0 (single group) last for
            # a short tail; DMAs staged in first-need order on two queues.
            SLOT_ORDER = list(range(NSLOT - 1, -1, -1))
            nc.sync.dma_start(out=kT_sb[:, 0:2], in_=kT_dr[:, 0:2])
            nc.sync.dma_start(out=qT_sb[:, 7:8], in_=qT_dr[:, 7:8])
            nc.sync.dma_start(out=v_sb[:, 0:16], in_=v_dr[:, 0:16])
            nc.sync.dma_start(out=kT_sb[:, 2:8], in_=kT_dr[:, 2:8])
            nc.gpsimd.dma_start(out=mask_sb[:], in_=m_d.rearrange("p (j q) -> p j q", q=QT))
            nc.sync.dma_start(out=kT_sb[:, 8:16], in_=kT_dr[:, 8:16])
            nc.gpsimd.dma_start(out=v_sb[:, 16:32], in_=v_dr[:, 16:32])
            nc.sync.dma_start(out=qT_sb[:, 0:7], in_=qT_dr[:, 0:7])

            # Software pipelining, two levels:
            #  - each group's PV is emitted after the NEXT group's QK+exp so
            #    the in-order PE queue never head-of-line blocks on ACT
            #  - each slot's masked-group PV + output copy/DMA (finalize) is
            #    deferred past the next slot's first QK+exp
            pending_finalize = [None]
            pending_pv = [None]

            def emit_pv(acc, pT, g, start, stop):
                for j in range(4):
                    kb = 4 * g + PERM[j]
                    nc.tensor.matmul(
                        acc[:], v_sb[:, kb, :], pT[:, j, :],
                        start=start and j == 0, stop=stop and j == 3,
                    )

            for t in SLOT_ORDER:
                ngroup = t + 1
                acc = accpool.tile([65, QT], F32)
                for g in range(ngroup):
                    masked = g == ngroup - 1
                    sc = scpool.tile([128, 4, QT], F32)
                    # two concurrent row-packed matmuls write different banks:
                    # positions (0,2) = bank0/bank1, then (1,3)
                    for half in (0, 1):
                        nc.tensor.matmul(
                            sc[:, half, :],
                            kT_sb[0:64, 2 * g + half, :],
                            qT_sb[0:64, t, :],
                            start=True, stop=True,
                        )
                        nc.tensor.matmul(
                            sc[:, half + 2, :],
                            kT_sb[64:128, 2 * g + half, :],
                            qT_sb[64:128, t, :],
                            start=True, stop=True,
                        )
                    # masked groups get their own slots so DVE-written buffers
                    # never alias the plain exp pipeline (avoids ACT waits)
                    pool = mpool if masked else ppool
                    pT = pool.tile([128, 4, QT], BF16)
                    nc.scalar.activation(
                        out=pT[:], in_=sc[:], bias=zbias[:],
                        func=mybir.ActivationFunctionType.Exp, scale=SCALE,
                    )
                    if g == 0 and pending_finalize[0] is not None:
                        pending_finalize[0]()
                        pending_finalize[0] = None
                    if pending_pv[0] is not None:
                        pending_pv[0]()
                        pending_pv[0] = None
                    if masked:
                        nc.vector.tensor_mul(
                            pT.rearrange("p a q -> p (a q)"),
                            pT.rearrange("p a q -> p (a q)"),
                            mask_sb.rearrange("p a q -> p (a q)"),
                        )

                        def finalize(acc=acc, pT=pT, g=g, t=t, first=(ngroup == 1)):
                            emit_pv(acc, pT, g, start=first, stop=True)
                            o_sb = opool.tile([65, QT], F32)
                            nc.vector.tensor_copy(o_sb[:], acc[:])
                            nc.sync.dma_start(out=out_d[:, t, :], in_=o_sb[:])

                        pending_finalize[0] = finalize
                    else:
                        def pv(acc=acc, pT=pT, g=g, first=(g == 0)):
                            emit_pv(acc, pT, g, start=first, stop=False)

                        pending_pv[0] = pv
            pending_finalize[0]()
    return nc


def _host_inputs(q, k, v):
    """Build per-core device input maps. q,k,v: [B,S,D] float32."""
    tri0 = (np.arange(QT)[None, :] >= np.arange(128)[:, None]).astype(npbf16)
    tri128 = (np.arange(QT)[None, :] >= (np.arange(128)[:, None] + 128)).astype(npbf16)
    ones = np.ones((128, QT), npbf16)
    zero = np.zeros((128, QT), npbf16)
    # position j holds block 4t+PERM[j]; PERM=(0,2,1,3)
    # type A (tile 2t, nk=4t+2): blocks 4t,4t+1 diag; 4t+2,4t+3 pad
    #   -> positions [TRI0, ZERO, TRI128, ZERO]
    # type B (tile 2t+1, nk=4t+4): blocks 4t,4t+1 full; 4t+2,4t+3 diag
    #   -> positions [ONES, TRI0, ONES, TRI128]
    mA = np.stack([tri0, zero, tri128, zero])  # [4,128,QT]
    mB = np.stack([ones, tri0, ones, tri128])
    dmasks = [m.transpose(1, 0, 2).reshape(128, -1).copy() for m in (mA, mB)]

    in_maps = []
    for c in range(NCORES):
        b, ty = c // 2, c % 2
        qb = q[b].astype(npbf16)
        kb = k[b].astype(npbf16)
        vb = v[b].astype(npbf16)
        # qT: [128, 8*256], slot t = Q[tile rows].T duplicated in both halves
        qT = np.empty((128, NSLOT * QT), npbf16)
        for t in range(NSLOT):
            i = 2 * t + ty
            blk = qb[i * QT:(i + 1) * QT, :].T  # [64, 256]
            qT[0:64, t * QT:(t + 1) * QT] = blk
            qT[64:128, t * QT:(t + 1) * QT] = blk
        # kT: pair p -> partitions 0:64 = K^T block 2p, 64:128 = block 2p+1
        kT = kb.reshape(16, 2, KB, D).transpose(1, 3, 0, 2).reshape(128, 16 * KB).copy()
        # v': [S, 65] with ones column, laid out [128, 32*65]
        va = np.concatenate([vb, np.ones((S, 1), npbf16)], axis=1)
        vdev = va.reshape(32, KB, 65).transpose(1, 0, 2).reshape(128, 32 * 65).copy()
        in_maps.append({
            "qT": np.ascontiguousarray(qT),
            "kT": np.ascontiguousarray(kT),
            "v": np.ascontiguousarray(vdev),
            "dmask": np.ascontiguousarray(dmasks[ty]),
        })
    return in_maps


_LAST_PERF = {}


def kernel(q, k, v, causal, trace=False):
    q = np.asarray(q, np.float32)
    k = np.asarray(k, np.float32)
    v = np.asarray(v, np.float32)
    assert int(causal) == 1
    nc = build_nc()
    in_maps = _host_inputs(q, k, v)
    res = run_bass_kernel_spmd(nc, in_maps, core_ids=list(range(NCORES)), trace=trace)
    _LAST_PERF["exec_time_ns"] = res.exec_time_ns
    _LAST_PERF["trace"] = res.instructions_and_trace
    _LAST_PERF["mean_exec_time_ns"] = res.mean_exec_time_ns

    out = np.empty((B, S, D), np.float32)
    for c in range(NCORES):
        b, ty = c // 2, c % 2
        oT = res.results[c]["outT"]  # [65, 8, 256] f32
        for t in range(NSLOT):
            i = 2 * t + ty
            o = oT[0:64, t, :] / oT[64:65, t, :]
            out[b, i * QT:(i + 1) * QT, :] = o.T
    return out


# revision 27
# speedup vs baseline: 1.0199x; 1.0199x over previous
"""Causal attention B=4 S=4096 D=64 on 8 TRN2 NeuronCores.

Sharding: core c -> batch b = c//2, type = c%2. Each core owns 8 q-tiles of
256 rows: type A takes tiles i=2t, type B i=2t+1 (t=0..7), so every core's
causal work is ~balanced and the compiled program is identical across cores
(slot t processes cap_t = 4t+4 k-blocks of 128 rows; cores whose tile needs
fewer blocks have the excess masked to zero via per-core mask data).

Device algorithm per (slot t, group g of 4 k-blocks):
  S^T[k,q] = K_blk @ Q^T  (two K=64-contraction matmuls row-packed via
  partitions 0:64 / 64:128, outputs in different PSUM banks), exp(S^T/8)
  fused on ScalarE PSUM->SBUF bf16, last group multiplied by per-core mask
  (causal triangle / pad zeroing), then out^T[65,256] += V'_blk.T @ P^T
  accumulated in PSUM where V' has a ones column so row 64 = softmax
  denominator. Normalize + transpose on host.
"""

import json

import numpy as np
import ml_dtypes

import concourse.bass as bass
import concourse.mybir as mybir
import concourse.tile as tile
from concourse.bass_utils import run_bass_kernel_spmd
from concourse.vector_clock import ScopedClock

B, S, D = 4, 4096, 64
NCORES = 8
QT = 256               # q-tile width
NSLOT = 8              # q-tiles per core
KB = 128               # k-block rows
SCALE = 1.0 / 8.0      # 1/sqrt(D)
PERM = (0, 2, 1, 3)    # sc position j holds block 4g + PERM[j]

BF16 = mybir.dt.bfloat16
F32 = mybir.dt.float32
npbf16 = ml_dtypes.bfloat16


def _patched_drain_and_barrier(self, tick_clock, wait_clock):
    """The tail Drain may carry N sem waits; this walrus build rejects >1
    wait on CTRL-class instructions, so split them across N drains."""
    drain_inst = self.nc.sync.drain()
    wait_clock.add_sem_waits(
        drain_inst.ins, ScopedClock({None: tick_clock.global_clock})
    )
    si = drain_inst.ins.sync_info
    waits = list(si.on_wait) if si and si.on_wait else []
    if len(waits) > 1:
        si.on_wait = waits[:1]
        for w in waits[1:]:
            d2 = self.nc.sync.drain()
            si2 = d2.ins.sync_info
            if si2 is None:
                d2.ins.sync_info = mybir.SyncInfo(on_wait=[w], on_update=[])
            else:
                si2.on_wait = [w]
    self.nc.all_engine_barrier()
    popped = self.nc._tile_sem_poison_stack.pop()
    assert popped is self._sem_poison
    self.nc.clear_and_free_semaphores(list(self.sems.allocated().values()))
    self.nc.all_engine_barrier()


tile.TileContext._drain_and_barrier = _patched_drain_and_barrier

_orig_to_json_bytes = bass.Bass.to_json_bytes


def _to_json_bytes_split_waits(self) -> bytes:
    """This walrus build accepts at most one sem wait per instruction; spill
    extra waits onto standalone EventSemaphore instructions just before."""
    m = json.loads(_orig_to_json_bytes(self))
    ctr = 0
    for fn in m["functions"]:
        for blk in fn["blocks"]:
            if blk["name"] == "main":
                blk["instructions"] = [
                    i for i in blk["instructions"]
                    if i["opcode"] not in ("Memset", "Drain", "EventSemaphore")
                ]
        for blk in fn["blocks"]:
            out = []
            for inst in blk["instructions"]:
                si = inst.get("sync_info")
                ow = (si or {}).get("on_wait") or []
                if ow:
                    # engines execute their queue in order, so a wait on the
                    # instruction's own engine-completion semaphore is always
                    # already satisfied — drop it (DMA queue sems are named
                    # DMAHW*/DMASW* and never match the engine string).
                    eng = inst.get("engine")
                    kept = [
                        w for w in ow
                        if w.get("ant_name", "").rsplit("_", 1)[0] != eng
                    ]
                    if len(kept) != len(ow):
                        si["on_wait"] = ow = kept
                if len(ow) > 1:
                    for w in ow[:-1]:
                        ctr += 1
                        out.append({
                            "debug": inst.get("debug", 0),
                            "engine": inst["engine"],
                            "ins": [],
                            "outs": [],
                            "name": f"{inst['name']}_sw{ctr}",
                            "opcode": "EventSemaphore",
                            "sync_info": {"on_update": [], "on_wait": [w]},
                        })
                    si["on_wait"] = [ow[-1]]
                out.append(inst)
            blk["instructions"] = out
    return json.dumps(m).encode()


bass.Bass.to_json_bytes = _to_json_bytes_split_waits


def build_nc():
    nc = bass.Bass()
    qT_d = nc.declare_dram_parameter("qT", [128, NSLOT * QT], BF16, isOutput=False)
    kT_d = nc.declare_dram_parameter("kT", [128, 16 * KB], BF16, isOutput=False)
    v_d = nc.declare_dram_parameter("v", [128, 32 * 65], BF16, isOutput=False)
    m_d = nc.declare_dram_parameter("dmask", [128, 4 * QT], BF16, isOutput=False)
    out_d = nc.declare_dram_parameter("outT", [65, NSLOT, QT], F32, isOutput=True)

    with tile.TileContext(nc) as tc:
        with (
            tc.tile_pool(name="const", bufs=1) as cpool,
            tc.tile_pool(name="pT", bufs=6) as ppool,
            tc.tile_pool(name="pTm", bufs=2) as mpool,
            tc.tile_pool(name="osb", bufs=3) as opool,
            tc.tile_pool(name="sc", bufs=3, space="PSUM") as scpool,
            tc.tile_pool(name="acc", bufs=2, space="PSUM") as accpool,
        ):
            # explicit zero bias (avoids the const_aps preamble) + warm
            # the ACT exp table during the DMA prefetch window
            zbias = cpool.tile([128, 1], F32)
            nc.vector.memset(zbias[:], 0.0)
            warm = cpool.tile([128, 8], F32)
            nc.vector.memset(warm[:], 0.0)
            nc.scalar.activation(out=warm[:], in_=warm[:], bias=zbias[:],
                                 func=mybir.ActivationFunctionType.Exp)



            qT_sb = cpool.tile([128, NSLOT, QT], BF16)
            kT_sb = cpool.tile([128, 16, KB], BF16)
            v_sb = cpool.tile([128, 32, 65], BF16)
            mask_sb = cpool.tile([128, 4, QT], BF16)
            qT_dr = qT_d.rearrange("p (s q) -> p s q", q=QT)
            kT_dr = kT_d.rearrange("p (b k) -> p b k", k=KB)
            v_dr = v_d.rearrange("p (b v) -> p b v", v=65)
            # big slots first (descending), slot 0 (single group) last for
            # a short tail; DMAs staged in first-need order on two queues.
            SLOT_ORDER = list(range(NSLOT - 1, -1, -1))
            nc.sync.dma_start(out=kT_sb[:, 0:2], in_=kT_dr[:, 0:2])
            nc.sync.dma_start(out=qT_sb[:, 7:8], in_=qT_dr[:, 7:8])
            nc.sync.dma_start(out=kT_sb[:, 2:8], in_=kT_dr[:, 2:8])
            nc.sync.dma_start(out=v_sb[:, 0:16], in_=v_dr[:, 0:16])
            nc.gpsimd.dma_start(out=mask_sb[:], in_=m_d.rearrange("p (j q) -> p j q", q=QT))
            nc.sync.dma_start(out=kT_sb[:, 8:16], in_=kT_dr[:, 8:16])
            nc.gpsimd.dma_start(out=v_sb[:, 16:32], in_=v_dr[:, 16:32])
            nc.sync.dma_start(out=qT_sb[:, 0:7], in_=qT_dr[:, 0:7])

            # Software pipelining, two levels:
            #  - each group's PV is emitted after the NEXT group's QK+exp so
            #    the in-order PE queue never head-of-line blocks on ACT
            #  - each slot's masked-group PV + output copy/DMA (finalize) is
            #    deferred past the next slot's first QK+exp
            pending_finalize = [None]
            pending_pv = [None]

            def emit_pv(acc, pT, g, start, stop):
                for j in range(4):
                    kb = 4 * g + PERM[j]
                    nc.tensor.matmul(
                        acc[:], v_sb[:, kb, :], pT[:, j, :],
                        start=start and j == 0, stop=stop and j == 3,
                    )

            for t in SLOT_ORDER:
                ngroup = t + 1
                acc = accpool.tile([65, QT], F32)
                for g in range(ngroup):
                    masked = g == ngroup - 1
                    sc = scpool.tile([128, 4, QT], F32)
                    # two concurrent row-packed matmuls write different banks:
                    # positions (0,2) = bank0/bank1, then (1,3)
                    for half in (0, 1):
                        nc.tensor.matmul(
                            sc[:, half, :],
                            kT_sb[0:64, 2 * g + half, :],
                            qT_sb[0:64, t, :],
                            start=True, stop=True,
                        )
                        nc.tensor.matmul(
                            sc[:, half + 2, :],
                            kT_sb[64:128, 2 * g + half, :],
                            qT_sb[64:128, t, :],
                            start=True, stop=True,
                        )
                    # masked groups get their own slots so DVE-written buffers
                    # never alias the plain exp pipeline (avoids ACT waits)
                    pool = mpool if masked else ppool
                    pT = pool.tile([128, 4, QT], BF16)
                    nc.scalar.activation(
                        out=pT[:], in_=sc[:], bias=zbias[:],
                        func=mybir.ActivationFunctionType.Exp, scale=SCALE,
                    )
                    if g == 0 and pending_finalize[0] is not None:
                        pending_finalize[0]()
                        pending_finalize[0] = None
                    if pending_pv[0] is not None:
                        pending_pv[0]()
                        pending_pv[0] = None
                    if masked:
                        nc.vector.tensor_mul(
                            pT.rearrange("p a q -> p (a q)"),
                            pT.rearrange("p a q -> p (a q)"),
                            mask_sb.rearrange("p a q -> p (a q)"),
                        )

                        def finalize(acc=acc, pT=pT, g=g, t=t, first=(ngroup == 1)):
                            emit_pv(acc, pT, g, start=first, stop=True)
                            o_sb = opool.tile([65, QT], F32)
                            nc.vector.tensor_copy(o_sb[:], acc[:])
                            nc.sync.dma_start(out=out_d[:, t, :], in_=o_sb[:])

                        pending_finalize[0] = finalize
                    else:
                        def pv(acc=acc, pT=pT, g=g, first=(g == 0)):
                            emit_pv(acc, pT, g, start=first, stop=False)

                        pending_pv[0] = pv
            pending_finalize[0]()
    return nc


def _host_inputs(q, k, v):
    """Build per-core device input maps. q,k,v: [B,S,D] float32."""
    tri0 = (np.arange(QT)[None, :] >= np.arange(128)[:, None]).astype(npbf16)
    tri128 = (np.arange(QT)[None, :] >= (np.arange(128)[:, None] + 128)).astype(npbf16)
    ones = np.ones((128, QT), npbf16)
    zero = np.zeros((128, QT), npbf16)
    # position j holds block 4t+PERM[j]; PERM=(0,2,1,3)
    # type A (tile 2t, nk=4t+2): blocks 4t,4t+1 diag; 4t+2,4t+3 pad
    #   -> positions [TRI0, ZERO, TRI128, ZERO]
    # type B (tile 2t+1, nk=4t+4): blocks 4t,4t+1 full; 4t+2,4t+3 diag
    #   -> positions [ONES, TRI0, ONES, TRI128]
    mA = np.stack([tri0, zero, tri128, zero])  # [4,128,QT]
    mB = np.stack([ones, tri0, ones, tri128])
    dmasks = [m.transpose(1, 0, 2).reshape(128, -1).copy() for m in (mA, mB)]

    in_maps = []
    for c in range(NCORES):
        b, ty = c // 2, c % 2
        qb = q[b].astype(npbf16)
        kb = k[b].astype(npbf16)
        vb = v[b].astype(npbf16)
        # qT: [128, 8*256], slot t = Q[tile rows].T duplicated in both halves
        qT = np.empty((128, NSLOT * QT), npbf16)
        for t in range(NSLOT):
            i = 2 * t + ty
            blk = qb[i * QT:(i + 1) * QT, :].T  # [64, 256]
            qT[0:64, t * QT:(t + 1) * QT] = blk
            qT[64:128, t * QT:(t + 1) * QT] = blk
        # kT: pair p -> partitions 0:64 = K^T block 2p, 64:128 = block 2p+1
        kT = kb.reshape(16, 2, KB, D).transpose(1, 3, 0, 2).reshape(128, 16 * KB).copy()
        # v': [S, 65] with ones column, laid out [128, 32*65]
        va = np.concatenate([vb, np.ones((S, 1), npbf16)], axis=1)
        vdev = va.reshape(32, KB, 65).transpose(1, 0, 2).reshape(128, 32 * 65).copy()
        in_maps.append({
            "qT": np.ascontiguousarray(qT),
            "kT": np.ascontiguousarray(kT),
            "v": np.ascontiguousarray(vdev),
            "dmask": np.ascontiguousarray(dmasks[ty]),
        })
    return in_maps


_LAST_PERF = {}


def kernel(q, k, v, causal, trace=False):
    q = np.asarray(q, np.float32)
    k = np.asarray(k, np.float32)
    v = np.asarray(v, np.float32)
    assert int(causal) == 1
    nc = build_nc()
    in_maps = _host_inputs(q, k, v)
    res = run_bass_kernel_spmd(nc, in_maps, core_ids=list(range(NCORES)), trace=trace)
    _LAST_PERF["exec_time_ns"] = res.exec_time_ns
    _LAST_PERF["trace"] = res.instructions_and_trace
    _LAST_PERF["mean_exec_time_ns"] = res.mean_exec_time_ns

    out = np.empty((B, S, D), np.float32)
    for c in range(NCORES):
        b, ty = c // 2, c % 2
        oT = res.results[c]["outT"]  # [65, 8, 256] f32
        for t in range(NSLOT):
            i = 2 * t + ty
            o = oT[0:64, t, :] / oT[64:65, t, :]
            out[b, i * QT:(i + 1) * QT, :] = o.T
    return out
